# revision 53
# baseline (speedup 1.0000x reference)
"""Causal self-attention (B=4, T=2048, C=2048, H=16) on 8 NeuronCores.

Sharding: core c = (b, g) with b = c // 2 (batch), g = c % 2 (head group of 8
heads = 1024 channels). Data parallel over B, tensor parallel over heads; the
output projection is computed per head-group and the two partials per batch
are summed on the host (+ bp).

Device program: a fused per-head pipeline. The QKV projections for head h+1
(pure GEMM) are interleaved into head h's attention steps so the scalar
(exp) and vector (sums/normalize) work hides under tensor-engine GEMMs and
the PE never idles long enough to re-throttle (HAM).

Per chunk step s = (h, c) with njb = 4(c+1) key blocks:
  PE:  S^T blocks (keys on partitions)  ->  AV accumulation of chunk s-1
       -> Z broadcast matmuls (ones128^T @ za/zb) -> projection GEMM slice
       (q/k for head h+1, v for head-pair h//2+1)
  ACT: exp of each S block with the additive attn mask as per-partition bias
  DVE: 1/Z (128-wide, in SBUF), yT normalize mult of chunk s-1, causal
       staircase zeroing (one bf16 0/1 multiply per chunk), the za/zb
       pairwise partial-sum tree, projection bias adds.
v stays resident in SBUF (no DRAM round trip); Z broadcast comes straight
from a [128,128] ones stationary so there is no transpose/DRAM bounce on the
PE critical path. Phase 3 (out = yT^T Wp) streams Wp after the slots finish.
"""

import math

import numpy as np
import ml_dtypes

import concourse.bass as bass
import concourse.bacc as bacc
import concourse.mybir as mybir
from concourse.tile import TileContext
from concourse.bass_utils import run_bass_kernel_spmd

T = 2048
C = 2048
N_HEAD = 16
D = 128          # head dim
HG = 8           # heads per core
CG = HG * D      # 1024: per-core projection width
B = 4
N_CORES = 8

F32 = mybir.dt.float32
BF16 = mybir.dt.bfloat16

_NC_CACHE = None


def _build_program():
    nc = bacc.Bacc("TRN2", target_bir_lowering=False, debug=False)

    # inputs arrive pre-transposed on the host into the exact SBUF layouts
    # so every DMA is fully contiguous (4KB+ runs per partition)
    x4 = nc.dram_tensor("x4", [128, 4, 16, 512], BF16, kind="ExternalInput")
    wq4 = nc.dram_tensor("wq4", [128, HG, 16, 128], BF16,
                         kind="ExternalInput")
    wk4 = nc.dram_tensor("wk4", [128, HG, 16, 128], BF16,
                         kind="ExternalInput")
    wv4 = nc.dram_tensor("wv4", [128, 4, 16, 256], BF16,
                         kind="ExternalInput")
    bq = nc.dram_tensor("bq", [128, HG], F32, kind="ExternalInput")
    bk = nc.dram_tensor("bk", [128, HG], F32, kind="ExternalInput")
    bvb = nc.dram_tensor("bvb", [128, CG], BF16, kind="ExternalInput")
    wpT = nc.dram_tensor("wpT", [CG, C], BF16, kind="ExternalInput")
    maskT = nc.dram_tensor("maskT", [128, 16], F32, kind="ExternalInput")
    cdg01 = nc.dram_tensor("cdg01", [128, 4, 512], BF16, kind="ExternalInput")
    ones128 = nc.dram_tensor("ones128", [128, 128], BF16, kind="ExternalInput")
    out = nc.dram_tensor("out", [T, C], BF16, kind="ExternalOutput")

    add = mybir.AluOpType.add
    mult = mybir.AluOpType.mult
    Exp = mybir.ActivationFunctionType.Exp
    Copy = mybir.ActivationFunctionType.Copy

    with TileContext(nc) as tc:
        # ---- constants that live for the whole kernel ----
        with tc.tile_pool(name="const", bufs=1) as cpool:
            # the tiny q/k/v biases gate the first projection drains (~27us):
            # allocated here, but their DMAs are issued in the prologue right
            # after the first weights so they neither stall the drain chain
            # nor delay the first x/weight transfers
            bq_sb = cpool.tile([128, HG], F32)
            bk_sb = cpool.tile([128, HG], F32)
            bv_sb = cpool.tile([128, CG], BF16)
            maskT_sb = cpool.tile([128, 16], F32)
            cdg01_sb = cpool.tile([128, 4, 512], BF16)
            ones_sb = cpool.tile([128, 128], BF16)

            with tc.tile_pool(name="yt", bufs=1) as ytpool:
                yT_sb = ytpool.tile([128, HG, T], BF16)

                with (
                    tc.tile_pool(name="xx", bufs=1) as xpool,
                    tc.tile_pool(name="qk", bufs=2) as qkpool,
                    tc.tile_pool(name="vh", bufs=2) as vhpool,
                    tc.tile_pool(name="wv", bufs=2) as wvpool,
                    tc.tile_pool(name="wqk", bufs=2) as wqkpool,
                    tc.tile_pool(name="pt", bufs=2) as ptpool,
                    tc.tile_pool(name="zz", bufs=2) as zpool,
                    tc.tile_pool(name="psqk", bufs=2, space="PSUM") as psqk,
                    tc.tile_pool(name="psv", bufs=2, space="PSUM") as psv,
                    tc.tile_pool(name="psst", bufs=3, space="PSUM") as psst,
                    tc.tile_pool(name="psy", bufs=1, space="PSUM") as psy,
                ):
                    # ---------- startup DMAs across 4 queues ----------
                    # x loaded in t-slices so the tr-major prologue can start
                    # on slice 0 while the rest stream in. Slice 0 goes first
                    # on an otherwise-empty queue, split in cc halves so the
                    # first projection matmuls can start on the first half.
                    xt = xpool.tile([128, 4, 16, 512], BF16)

                    def dma_x(ts, clo, chi, queue):
                        queue.dma_start(
                            out=xt[:, ts, clo:chi, :],
                            in_=x4[:, ts, clo:chi, :],
                        )

                    dma_x(0, 0, 8, nc.sync)
                    dma_x(0, 8, 16, nc.scalar)
                    dma_x(1, 0, 8, nc.sync)
                    dma_x(1, 8, 16, nc.scalar)

                    wq_t = {}   # (head, 'q'/'k') -> weight tile
                    wv_t = {}   # pair -> weight tile
                    q_ring = {}
                    k_ring = {}
                    vh_ring = {}

                    def dma_wqk(dc, which, queue):
                        w_dram = wq4 if which == "q" else wk4
                        wt = wqkpool.tile([128, 16, 128], BF16,
                                          tag=f"w{which}",
                                          name=f"w{which}{dc}")
                        queue.dma_start(out=wt, in_=w_dram[:, dc, :, :])
                        wq_t[(dc, which)] = wt

                    def dma_wv(p, queue):
                        wt = wvpool.tile([128, 16, 256], BF16, tag="wv",
                                         name=f"wv{p}")
                        queue.dma_start(out=wt, in_=wv4[:, p, :, :])
                        wv_t[p] = wt

                    def qk_quarter(dc, which, tr):
                        """Thunks for 16 matmuls + 1 bias drain: one 512-t
                        quarter of q or k for head dc."""
                        ring = q_ring if which == "q" else k_ring
                        if dc not in ring:
                            ring[dc] = qkpool.tile(
                                [128, T], BF16, tag=f"{which}ring",
                                name=f"{which}{dc}",
                            )
                        wt = wq_t[(dc, which)]
                        b_sb = bq_sb if which == "q" else bk_sb
                        ps = psqk.tile([128, 512], F32, tag="qkps",
                                       name="qkps")

                        def mm(cc):
                            nc.tensor.matmul(
                                ps,
                                wt[:, cc, :],
                                xt[:, tr, cc, :],
                                start=(cc == 0),
                                stop=(cc == 15),
                            )

                        def drain():
                            nc.vector.tensor_scalar_add(
                                ring[dc][:, tr * 512:(tr + 1) * 512],
                                ps, b_sb[:, dc:dc + 1],
                            )

                        return [lambda cc=cc: mm(cc) for cc in range(16)] + \
                            [drain]

                    def v_group(p, tcb):
                        """Thunks for 16 matmuls (N=256) + bias drain: one
                        t-block of v for head pair p."""
                        if p not in vh_ring:
                            vh_ring[p] = vhpool.tile(
                                [128, 16, 256], BF16, tag="vh", name=f"vh{p}",
                            )
                        ps = psv.tile([128, 256], F32, tag="vps", name="vps")
                        ts_, t_ = divmod(tcb, 4)

                        def mm(cc):
                            nc.tensor.matmul(
                                ps,
                                xt[:, ts_, cc, t_ * 128:(t_ + 1) * 128],
                                wv_t[p][:, cc, :],
                                start=(cc == 0),
                                stop=(cc == 15),
                            )

                        def drain():
                            nc.vector.tensor_tensor(
                                vh_ring[p][:, tcb, :], ps,
                                bv_sb[:, p * 256:(p + 1) * 256], add,
                            )

                        return [lambda cc=cc: mm(cc) for cc in range(16)] + \
                            [drain]

                    # ---------- prologue ----------
                    dma_wqk(0, "q", nc.gpsimd)
                    dma_wqk(0, "k", nc.gpsimd)
                    dma_wv(0, nc.gpsimd)
                    nc.gpsimd.dma_start(out=bq_sb, in_=bq[:, :])
                    nc.gpsimd.dma_start(out=bk_sb, in_=bk[:, :])
                    nc.gpsimd.dma_start(out=bv_sb, in_=bvb[:, :])
                    dma_x(2, 0, 16, nc.gpsimd)
                    dma_wqk(1, "q", nc.scalar)
                    nc.scalar.dma_start(out=maskT_sb, in_=maskT[:, :])
                    nc.scalar.dma_start(out=cdg01_sb, in_=cdg01[:, :, :])
                    nc.scalar.dma_start(out=ones_sb, in_=ones128[:, :])
                    dma_x(3, 0, 16, nc.scalar)
                    dma_wv(1, nc.gpsimd)
                    dma_wqk(1, "k", nc.gpsimd)

                    # tr-major so compute starts on x t-slice 0 immediately;
                    # tr0 is interleaved at cc-half granularity so the first
                    # matmuls start on the first half of x slice 0
                    q0 = qk_quarter(0, "q", 0)
                    k0 = qk_quarter(0, "k", 0)
                    vg = [v_group(0, tcb) for tcb in range(4)]
                    for t in q0[0:8] + k0[0:8] + vg[0][0:8] + vg[1][0:8]:
                        t()
                    for t in q0[8:] + k0[8:] + vg[0][8:] + vg[1][8:]:
                        t()
                    for t in vg[2] + vg[3]:
                        t()
                    for tr in range(1, 4):
                        for t in qk_quarter(0, "q", tr):
                            t()
                        for t in qk_quarter(0, "k", tr):
                            t()
                        for tcb in range(4 * tr, 4 * tr + 4):
                            for t in v_group(0, tcb):
                                t()

                    # qk projection schedule: step -> [(dc, which, tr)].
                    # Head 7's k quarters 2/3 are deferred into slot 7 so its
                    # steps have PE filler.
                    qk_sched = {}
                    for hh in range(6):
                        for cc_ in range(4):
                            qk_sched[4 * hh + cc_] = [
                                (hh + 1, "q", cc_), (hh + 1, "k", cc_)]
                    qk_sched[24] = [(7, "q", 0), (7, "k", 0)]
                    qk_sched[25] = [(7, "q", 1), (7, "k", 1)]
                    qk_sched[26] = [(7, "q", 2)]
                    qk_sched[27] = [(7, "q", 3)]
                    qk_sched[28] = [(7, "k", 2)]
                    qk_sched[29] = [(7, "k", 3)]

                    # ---------- fused head/chunk steps ----------
                    # per-chunk state kept across steps for the s-1 tail
                    state = {}

                    def av_thunks(h, c, pt, njb, za):
                        """Thunks: AV accumulation, Z broadcast matmul, then
                        1/Z + yT normalize (DVE) for chunk (h, c)."""
                        vh = vh_ring[h // 2]
                        dlo = (h % 2) * 128
                        yps = psy.tile([128, 512], F32, tag="y", name="y")
                        zbc = psv.tile([128, 512], F32, tag="vps", name="zbc")

                        def av_mm(jb):
                            sdg = jb - 4 * c
                            lo = sdg * 128 if sdg > 0 else 0
                            nc.tensor.matmul(
                                yps[:, lo:512],
                                vh[:, jb, dlo:dlo + 128],
                                pt[:, jb, lo:512],
                                start=(jb == 0),
                                stop=(jb == njb - 1),
                            )

                        def zm_norm():
                            nc.tensor.matmul(zbc, ones_sb, za, start=True,
                                             stop=True)
                            rr = zpool.tile([128, 512], F32, tag="rr",
                                            name="rr")
                            nc.vector.reciprocal_approx_fast(out=rr, in_=zbc)
                            nc.vector.tensor_tensor(
                                yT_sb[:, h, c * 512:(c + 1) * 512], yps, rr,
                                mult,
                            )

                        return [lambda jb=jb: av_mm(jb)
                                for jb in range(njb)] + [zm_norm]

                    for s in range(33):
                        # ---- build this step's filler (prev chunk tail +
                        # projection GEMMs) ----
                        filler = []
                        if s >= 1:
                            hp, cp = divmod(s - 1, 4)
                            ptp, njbp, zap = state.pop((hp, cp))
                            filler += av_thunks(hp, cp, ptp, njbp, zap)
                        if s < 32:
                            h, c = divmod(s, 4)
                            # weight prefetch for upcoming work
                            if c == 0 and h + 2 < HG:
                                dma_wqk(h + 2, "q", nc.sync)
                                dma_wqk(h + 2, "k", nc.gpsimd)
                            if c == 2 and h % 2 == 1:
                                p = (h + 3) // 2
                                if p <= 3:
                                    dma_wv(p, nc.gpsimd)
                            for (dc_, which_, tr_) in qk_sched.get(s, []):
                                filler += qk_quarter(dc_, which_, tr_)
                            p = h // 2 + 1
                            if p <= 3:
                                tb = (h % 2) * 8 + c * 2
                                filler += v_group(p, tb)
                                filler += v_group(p, tb + 1)

                        fi = iter(filler)

                        def pull(n):
                            for _ in range(n):
                                t = next(fi, None)
                                if t is None:
                                    return
                                t()

                        # ---- S blocks + exp, interleaved with filler ----
                        if s < 32:
                            njb = 4 * (c + 1)
                            pt = ptpool.tile([128, 16, 512], BF16, tag="pt")
                            # clear the stale [0, lo) regions of the diagonal
                            # blocks (read by the staircase multiply / AV)
                            for sdg in range(1, 4):
                                nc.vector.memset(
                                    pt[:, 4 * c + sdg, 0:sdg * 128], 0.0)
                            for jb in range(njb):
                                sdg = jb - 4 * c
                                lo = sdg * 128 if sdg > 0 else 0
                                # after the last projection (s >= 30) the qk
                                # PSUM banks are free: widen the S ring to 5
                                # so the exp lag never stalls the PE
                                if s >= 30 and jb % 5 >= 3:
                                    ps = psqk.tile([128, 512], F32,
                                                   tag="qkps", name="qkps")
                                else:
                                    ps = psst.tile([128, 512], F32, tag="s",
                                                   name="s")
                                nc.tensor.matmul(
                                    ps[:, lo:512],
                                    k_ring[h][:, jb * 128:(jb + 1) * 128],
                                    q_ring[h][:, c * 512 + lo:(c + 1) * 512],
                                    start=True,
                                    stop=True,
                                )
                                nc.scalar.activation(
                                    pt[:, jb, lo:512], ps[:, lo:512], Exp,
                                    bias=maskT_sb[:, jb:jb + 1],
                                )
                                pull(3)
                            # staircase zero of the diagonal blocks
                            nc.vector.tensor_tensor(
                                pt[:, 4 * c:4 * c + 4, :],
                                pt[:, 4 * c:4 * c + 4, :],
                                cdg01_sb[:, :, :], mult,
                            )
                            # pairwise partial-sum tree -> za, zb (bf16)
                            za = zpool.tile([128, 512], BF16, tag="za",
                                            name="za")
                            zb = zpool.tile([128, 512], BF16, tag="zb",
                                            name="zb")
                            nc.vector.tensor_tensor(za, pt[:, 0, :],
                                                    pt[:, 2, :], add)
                            nc.vector.tensor_tensor(zb, pt[:, 1, :],
                                                    pt[:, 3, :], add)
                            for base in range(4, njb, 2):
                                nc.vector.tensor_tensor(
                                    za, za, pt[:, base, :], add)
                                nc.vector.tensor_tensor(
                                    zb, zb, pt[:, base + 1, :], add)
                            # fold the two accumulator chains so the Z
                            # broadcast needs only one PE matmul
                            nc.vector.tensor_tensor(za, za, zb, add)
                            state[(h, c)] = (pt, njb, za)
                        # ---- flush remaining filler ----
                        pull(len(filler))

                # ---------- phase 3: out = yT^T @ WpT ----------
                with (
                    tc.tile_pool(name="wp", bufs=1) as wppool,
                    tc.tile_pool(name="p3ps", bufs=4, space="PSUM") as ps3,
                    tc.tile_pool(name="p3o", bufs=4) as op3,
                ):
                    wp_sb = wppool.tile([128, HG, C], BF16)
                    oq = [nc.sync, nc.scalar]
                    for hh in range(HG):
                        oq[hh % 2].dma_start(
                            out=wp_sb[:, hh, :],
                            in_=wpT[hh * 128:(hh + 1) * 128, :],
                        )
                    for tcb in range(16):
                        pss = [ps3.tile([128, 512], F32, tag="ps3",
                                        name=f"ps3_{cr}")
                               for cr in range(4)]
                        for hh in range(HG):
                            for cr in range(4):
                                nc.tensor.matmul(
                                    pss[cr],
                                    yT_sb[:, hh, tcb * 128:(tcb + 1) * 128],
                                    wp_sb[:, hh, cr * 512:(cr + 1) * 512],
                                    start=(hh == 0),
                                    stop=(hh == HG - 1),
                                )
                        for cr in range(4):
                            ob = op3.tile([128, 512], BF16, tag="ob")
                            if cr % 2 == 0:
                                nc.scalar.activation(ob, pss[cr], Copy)
                            else:
                                nc.vector.tensor_copy(ob, pss[cr])
                            (nc.sync if cr % 2 == 0 else nc.scalar).dma_start(
                                out=out[tcb * 128:(tcb + 1) * 128,
                                        cr * 512:(cr + 1) * 512],
                                in_=ob,
                            )
    nc.compile()
    return nc


def get_nc():
    global _NC_CACHE
    if _NC_CACHE is None:
        _NC_CACHE = _build_program()
    return _NC_CACHE


def prep_core_inputs(inputs):
    """Host-side sharding / layout prep: slice per (b, g), transpose to the
    layouts the device program wants, fold the 1/sqrt(d) softmax scale into
    Wq/bq."""
    f = lambda a: np.asarray(a, dtype=np.float32)
    bf = ml_dtypes.bfloat16
    x = f(inputs["x"])
    am = f(inputs["attn_mask"])
    Wq, bq_ = f(inputs["Wq"]), f(inputs["bq"])
    Wk, bk_ = f(inputs["Wk"]), f(inputs["bk"])
    Wv, bv_ = f(inputs["Wv"]), f(inputs["bv"])
    Wp = f(inputs["Wp"])
    scale = 1.0 / math.sqrt(D)

    # 0/1 staircase in S^T layout: for diagonal block s (0..3) of a 512-wide
    # query chunk, partition p = key offset within the 128-block, column
    # i_local in [0, 512): masked (dead) iff i_local < s*128 + p.
    ii = np.arange(512)[None, :]
    pp = np.arange(128)[:, None]
    cdg01_t = np.stack(
        [np.where(ii < s * 128 + pp, 0.0, 1.0) for s in range(4)], axis=1
    ).astype(bf)  # [128, 4, 512]

    # device DMA layouts: [partition, slice, cc, inner] so every transfer is
    # contiguous per partition
    def to4(wT, ns, ni):
        return np.ascontiguousarray(
            wT.reshape(16, 128, ns, ni).transpose(1, 2, 0, 3)).astype(bf)

    per_g = []
    for g in range(2):
        sl = slice(g * CG, (g + 1) * CG)
        per_g.append(dict(
            wq4=to4(Wq[sl].T * scale, HG, 128),
            wk4=to4(np.ascontiguousarray(Wk[sl].T), HG, 128),
            wv4=to4(np.ascontiguousarray(Wv[sl].T), 4, 256),
            bq=np.ascontiguousarray((bq_[sl] * scale).reshape(HG, 128).T),
            bk=np.ascontiguousarray(bk_[sl].reshape(HG, 128).T),
            bvb=np.ascontiguousarray(
                np.broadcast_to(bv_[sl], (128, CG))
            ).astype(bf),
            wpT=np.ascontiguousarray(Wp[:, sl].T).astype(bf),
        ))

    ones_t = np.ones((128, 128), dtype=bf)

    in_maps = []
    for core in range(N_CORES):
        b, g = core // 2, core % 2
        m = dict(per_g[g])
        m["x4"] = to4(x[b].T, 4, 512)
        m["maskT"] = np.ascontiguousarray(
            am[b, 0, 0, :].reshape(16, 128).T
        )
        m["cdg01"] = cdg01_t
        m["ones128"] = ones_t
        in_maps.append(m)
    return in_maps


def run(inputs, trace=False):
    nc = get_nc()
    in_maps = prep_core_inputs(inputs)
    rr = run_bass_kernel_spmd(nc, in_maps, list(range(N_CORES)), trace=trace)
    bp = np.asarray(inputs["bp"], dtype=np.float32)
    y = np.empty((B, T, C), dtype=np.float32)
    for b in range(B):
        y[b] = (np.asarray(rr.results[2 * b]["out"], dtype=np.float32)
                + np.asarray(rr.results[2 * b + 1]["out"], dtype=np.float32)
                + bp[None, :])
    return y, rr


def kernel(**inputs):
    y, _ = run(inputs)
    return y


# revision 55
# speedup vs baseline: 1.1845x; 1.1845x over previous
"""Causal self-attention (B=4, T=2048, C=2048, H=16) on 8 NeuronCores.

Sharding: core c = (b, g) with b = c // 2 (batch), g = c % 2 (head group of 8
heads = 1024 channels). Data parallel over B, tensor parallel over heads; the
output projection is computed per head-group and the two partials per batch
are summed on the host (+ bp).

Device program: a fused per-head pipeline. The QKV projections for head h+1
(pure GEMM) are interleaved into head h's attention steps so the scalar
(exp) and vector (sums/normalize) work hides under tensor-engine GEMMs and
the PE never idles long enough to re-throttle (HAM).

Per chunk step s = (h, c) with njb = 4(c+1) key blocks:
  PE:  S^T blocks (keys on partitions)  ->  AV accumulation of chunk s-1
       -> Z broadcast matmuls (ones128^T @ za/zb) -> projection GEMM slice
       (q/k for head h+1, v for head-pair h//2+1)
  ACT: exp of each S block with the additive attn mask as per-partition bias
  DVE: 1/Z (128-wide, in SBUF), yT normalize mult of chunk s-1, causal
       staircase zeroing (one bf16 0/1 multiply per chunk), the za/zb
       pairwise partial-sum tree, projection bias adds.
v stays resident in SBUF (no DRAM round trip); Z broadcast comes straight
from a [128,128] ones stationary so there is no transpose/DRAM bounce on the
PE critical path. Phase 3 (out = yT^T Wp) streams Wp after the slots finish.
"""

import math

import numpy as np
import ml_dtypes

import concourse.bass as bass
import concourse.bacc as bacc
import concourse.mybir as mybir
from concourse.tile import TileContext
from concourse.bass_utils import run_bass_kernel_spmd

T = 2048
C = 2048
N_HEAD = 16
D = 128          # head dim
HG = 8           # heads per core
CG = HG * D      # 1024: per-core projection width
B = 4
N_CORES = 8

F32 = mybir.dt.float32
BF16 = mybir.dt.bfloat16

_NC_CACHE = None


def _build_program():
    nc = bacc.Bacc("TRN2", target_bir_lowering=False, debug=False)

    # inputs arrive pre-transposed on the host into the exact SBUF layouts
    # so every DMA is fully contiguous (4KB+ runs per partition)
    x4 = nc.dram_tensor("x4", [128, 4, 16, 512], BF16, kind="ExternalInput")
    wq4 = nc.dram_tensor("wq4", [128, HG, 16, 128], BF16,
                         kind="ExternalInput")
    wk4 = nc.dram_tensor("wk4", [128, HG, 16, 128], BF16,
                         kind="ExternalInput")
    wv4 = nc.dram_tensor("wv4", [128, 4, 16, 256], BF16,
                         kind="ExternalInput")
    bq = nc.dram_tensor("bq", [128, HG], F32, kind="ExternalInput")
    bk = nc.dram_tensor("bk", [128, HG], F32, kind="ExternalInput")
    bvb = nc.dram_tensor("bvb", [128, CG], BF16, kind="ExternalInput")
    wpT = nc.dram_tensor("wpT", [CG, C], BF16, kind="ExternalInput")
    maskT = nc.dram_tensor("maskT", [128, 16], F32, kind="ExternalInput")
    cdg01 = nc.dram_tensor("cdg01", [128, 4, 512], BF16, kind="ExternalInput")
    ones128 = nc.dram_tensor("ones128", [128, 128], BF16, kind="ExternalInput")
    out = nc.dram_tensor("out", [T, C], BF16, kind="ExternalOutput")

    add = mybir.AluOpType.add
    mult = mybir.AluOpType.mult
    Exp = mybir.ActivationFunctionType.Exp
    Copy = mybir.ActivationFunctionType.Copy

    with TileContext(nc) as tc:
        # ---- constants that live for the whole kernel ----
        with tc.tile_pool(name="const", bufs=1) as cpool:
            # the tiny q/k/v biases gate the first projection drains (~27us):
            # allocated here, but their DMAs are issued in the prologue right
            # after the first weights so they neither stall the drain chain
            # nor delay the first x/weight transfers
            bq_sb = cpool.tile([128, HG], F32)
            bk_sb = cpool.tile([128, HG], F32)
            bv_sb = cpool.tile([128, CG], BF16)
            maskT_sb = cpool.tile([128, 16], F32)
            cdg01_sb = cpool.tile([128, 4, 512], BF16)
            ones_sb = cpool.tile([128, 128], BF16)

            with tc.tile_pool(name="yt", bufs=1) as ytpool:
                yT_sb = ytpool.tile([128, HG, T], BF16)

                with (
                    tc.tile_pool(name="xx", bufs=1) as xpool,
                    tc.tile_pool(name="qk", bufs=2) as qkpool,
                    tc.tile_pool(name="vh", bufs=2) as vhpool,
                    tc.tile_pool(name="wv", bufs=2) as wvpool,
                    tc.tile_pool(name="wqk", bufs=2) as wqkpool,
                    tc.tile_pool(name="pt", bufs=2) as ptpool,
                    tc.tile_pool(name="zz", bufs=2) as zpool,
                    tc.tile_pool(name="psqk", bufs=2, space="PSUM") as psqk,
                    tc.tile_pool(name="psv", bufs=2, space="PSUM") as psv,
                    tc.tile_pool(name="psst", bufs=3, space="PSUM") as psst,
                    tc.tile_pool(name="psy", bufs=1, space="PSUM") as psy,
                ):
                    # ---------- startup DMAs across 4 queues ----------
                    # x loaded in t-slices so the tr-major prologue can start
                    # on slice 0 while the rest stream in. Slice 0 goes first
                    # on an otherwise-empty queue, split in cc halves so the
                    # first projection matmuls can start on the first half.
                    xt = xpool.tile([128, 4, 16, 512], BF16)

                    def dma_x(ts, clo, chi, queue):
                        queue.dma_start(
                            out=xt[:, ts, clo:chi, :],
                            in_=x4[:, ts, clo:chi, :],
                        )

                    dma_x(0, 0, 8, nc.sync)
                    dma_x(0, 8, 16, nc.scalar)
                    dma_x(1, 0, 8, nc.sync)
                    dma_x(1, 8, 16, nc.scalar)

                    wq_t = {}   # (head, 'q'/'k') -> weight tile
                    wv_t = {}   # pair -> weight tile
                    q_ring = {}
                    k_ring = {}
                    vh_ring = {}

                    def dma_wqk(dc, which, queue):
                        w_dram = wq4 if which == "q" else wk4
                        wt = wqkpool.tile([128, 16, 128], BF16,
                                          tag=f"w{which}",
                                          name=f"w{which}{dc}")
                        queue.dma_start(out=wt, in_=w_dram[:, dc, :, :])
                        wq_t[(dc, which)] = wt

                    def dma_wv(p, queue):
                        wt = wvpool.tile([128, 16, 256], BF16, tag="wv",
                                         name=f"wv{p}")
                        queue.dma_start(out=wt, in_=wv4[:, p, :, :])
                        wv_t[p] = wt

                    def qk_quarter(dc, which, tr):
                        """Thunks for 16 matmuls + 1 bias drain: one 512-t
                        quarter of q or k for head dc."""
                        ring = q_ring if which == "q" else k_ring
                        if dc not in ring:
                            ring[dc] = qkpool.tile(
                                [128, T], BF16, tag=f"{which}ring",
                                name=f"{which}{dc}",
                            )
                        wt = wq_t[(dc, which)]
                        b_sb = bq_sb if which == "q" else bk_sb
                        ps = psqk.tile([128, 512], F32, tag="qkps",
                                       name="qkps")

                        def mm(cc):
                            nc.tensor.matmul(
                                ps,
                                wt[:, cc, :],
                                xt[:, tr, cc, :],
                                start=(cc == 0),
                                stop=(cc == 15),
                            )

                        def drain():
                            nc.vector.tensor_scalar_add(
                                ring[dc][:, tr * 512:(tr + 1) * 512],
                                ps, b_sb[:, dc:dc + 1],
                            )

                        return [lambda cc=cc: mm(cc) for cc in range(16)] + \
                            [drain]

                    def v_group(p, tcb):
                        """Thunks for 16 matmuls (N=256) + bias drain: one
                        t-block of v for head pair p."""
                        if p not in vh_ring:
                            vh_ring[p] = vhpool.tile(
                                [128, 16, 256], BF16, tag="vh", name=f"vh{p}",
                            )
                        ps = psv.tile([128, 256], F32, tag="vps", name="vps")
                        ts_, t_ = divmod(tcb, 4)

                        def mm(cc):
                            nc.tensor.matmul(
                                ps,
                                xt[:, ts_, cc, t_ * 128:(t_ + 1) * 128],
                                wv_t[p][:, cc, :],
                                start=(cc == 0),
                                stop=(cc == 15),
                            )

                        def drain():
                            nc.vector.tensor_tensor(
                                vh_ring[p][:, tcb, :], ps,
                                bv_sb[:, p * 256:(p + 1) * 256], add,
                            )

                        return [lambda cc=cc: mm(cc) for cc in range(16)] + \
                            [drain]

                    # ---------- prologue ----------
                    dma_wqk(0, "q", nc.gpsimd)
                    dma_wqk(0, "k", nc.gpsimd)
                    dma_wv(0, nc.gpsimd)
                    nc.gpsimd.dma_start(out=bq_sb, in_=bq[:, :])
                    nc.gpsimd.dma_start(out=bk_sb, in_=bk[:, :])
                    nc.gpsimd.dma_start(out=bv_sb, in_=bvb[:, :])
                    dma_x(2, 0, 16, nc.gpsimd)
                    dma_wqk(1, "q", nc.scalar)
                    nc.scalar.dma_start(out=maskT_sb, in_=maskT[:, :])
                    nc.scalar.dma_start(out=cdg01_sb, in_=cdg01[:, :, :])
                    nc.scalar.dma_start(out=ones_sb, in_=ones128[:, :])
                    dma_x(3, 0, 16, nc.scalar)
                    dma_wv(1, nc.gpsimd)
                    dma_wqk(1, "k", nc.gpsimd)

                    # tr-major so compute starts on x t-slice 0 immediately;
                    # tr0 is interleaved at cc-half granularity so the first
                    # matmuls start on the first half of x slice 0
                    q0 = qk_quarter(0, "q", 0)
                    k0 = qk_quarter(0, "k", 0)
                    vg = [v_group(0, tcb) for tcb in range(4)]
                    for t in q0[0:8] + k0[0:8] + vg[0][0:8] + vg[1][0:8]:
                        t()
                    for t in q0[8:] + k0[8:] + vg[0][8:] + vg[1][8:]:
                        t()
                    for t in vg[2] + vg[3]:
                        t()
                    for tr in range(1, 4):
                        for t in qk_quarter(0, "q", tr):
                            t()
                        for t in qk_quarter(0, "k", tr):
                            t()
                        for tcb in range(4 * tr, 4 * tr + 4):
                            for t in v_group(0, tcb):
                                t()

                    # qk projection schedule: step -> [(dc, which, tr)].
                    # Head 7's k quarters 2/3 are deferred into slot 7 so its
                    # steps have PE filler.
                    qk_sched = {}
                    for hh in range(6):
                        for cc_ in range(4):
                            qk_sched[4 * hh + cc_] = [
                                (hh + 1, "q", cc_), (hh + 1, "k", cc_)]
                    qk_sched[24] = [(7, "q", 0), (7, "k", 0)]
                    qk_sched[25] = [(7, "q", 1), (7, "k", 1)]
                    qk_sched[26] = [(7, "q", 2)]
                    qk_sched[27] = [(7, "q", 3)]
                    qk_sched[28] = [(7, "k", 2)]
                    qk_sched[29] = [(7, "k", 3)]

                    # ---------- fused head/chunk steps ----------
                    # per-chunk state kept across steps for the s-1 tail
                    state = {}

                    def av_thunks(h, c, pt, njb, za):
                        """Thunks: AV accumulation, Z broadcast matmul, then
                        1/Z + yT normalize (DVE) for chunk (h, c)."""
                        vh = vh_ring[h // 2]
                        dlo = (h % 2) * 128
                        yps = psy.tile([128, 512], F32, tag="y", name="y")
                        zbc = psv.tile([128, 512], F32, tag="vps", name="zbc")

                        def av_mm(jb):
                            sdg = jb - 4 * c
                            lo = sdg * 128 if sdg > 0 else 0
                            nc.tensor.matmul(
                                yps[:, lo:512],
                                vh[:, jb, dlo:dlo + 128],
                                pt[:, jb, lo:512],
                                start=(jb == 0),
                                stop=(jb == njb - 1),
                            )

                        def zm_norm():
                            nc.tensor.matmul(zbc, ones_sb, za, start=True,
                                             stop=True)
                            rr = zpool.tile([128, 512], F32, tag="rr",
                                            name="rr")
                            nc.vector.reciprocal_approx_fast(out=rr, in_=zbc)
                            nc.vector.tensor_tensor(
                                yT_sb[:, h, c * 512:(c + 1) * 512], yps, rr,
                                mult,
                            )

                        return [lambda jb=jb: av_mm(jb)
                                for jb in range(njb)] + [zm_norm]

                    for s in range(33):
                        # ---- build this step's filler (prev chunk tail +
                        # projection GEMMs) ----
                        filler = []
                        if s >= 1:
                            hp, cp = divmod(s - 1, 4)
                            ptp, njbp, zap = state.pop((hp, cp))
                            filler += av_thunks(hp, cp, ptp, njbp, zap)
                        if s < 32:
                            h, c = divmod(s, 4)
                            # weight prefetch for upcoming work
                            if c == 0 and h + 2 < HG:
                                dma_wqk(h + 2, "q", nc.sync)
                                dma_wqk(h + 2, "k", nc.gpsimd)
                            if c == 2 and h % 2 == 1:
                                p = (h + 3) // 2
                                if p <= 3:
                                    dma_wv(p, nc.gpsimd)
                            for (dc_, which_, tr_) in qk_sched.get(s, []):
                                filler += qk_quarter(dc_, which_, tr_)
                            p = h // 2 + 1
                            if p <= 3:
                                tb = (h % 2) * 8 + c * 2
                                filler += v_group(p, tb)
                                filler += v_group(p, tb + 1)

                        fi = iter(filler)

                        def pull(n):
                            for _ in range(n):
                                t = next(fi, None)
                                if t is None:
                                    return
                                t()

                        # ---- S blocks + exp, interleaved with filler ----
                        if s < 32:
                            njb = 4 * (c + 1)
                            pt = ptpool.tile([128, 16, 512], BF16, tag="pt")
                            # clear the stale [0, lo) regions of the diagonal
                            # blocks (read by the staircase multiply / AV)
                            for sdg in range(1, 4):
                                nc.vector.memset(
                                    pt[:, 4 * c + sdg, 0:sdg * 128], 0.0)
                            for jb in range(njb):
                                sdg = jb - 4 * c
                                lo = sdg * 128 if sdg > 0 else 0
                                # after the last projection (s >= 30) the qk
                                # PSUM banks are free: widen the S ring to 5
                                # so the exp lag never stalls the PE
                                if s >= 30 and jb % 5 >= 3:
                                    ps = psqk.tile([128, 512], F32,
                                                   tag="qkps", name="qkps")
                                else:
                                    ps = psst.tile([128, 512], F32, tag="s",
                                                   name="s")
                                nc.tensor.matmul(
                                    ps[:, lo:512],
                                    k_ring[h][:, jb * 128:(jb + 1) * 128],
                                    q_ring[h][:, c * 512 + lo:(c + 1) * 512],
                                    start=True,
                                    stop=True,
                                )
                                nc.scalar.activation(
                                    pt[:, jb, lo:512], ps[:, lo:512], Exp,
                                    bias=maskT_sb[:, jb:jb + 1],
                                )
                                pull(3)
                            # staircase zero of the diagonal blocks
                            nc.vector.tensor_tensor(
                                pt[:, 4 * c:4 * c + 4, :],
                                pt[:, 4 * c:4 * c + 4, :],
                                cdg01_sb[:, :, :], mult,
                            )
                            # pairwise partial-sum tree -> za, zb (bf16)
                            za = zpool.tile([128, 512], BF16, tag="za",
                                            name="za")
                            zb = zpool.tile([128, 512], BF16, tag="zb",
                                            name="zb")
                            nc.vector.tensor_tensor(za, pt[:, 0, :],
                                                    pt[:, 2, :], add)
                            nc.vector.tensor_tensor(zb, pt[:, 1, :],
                                                    pt[:, 3, :], add)
                            for base in range(4, njb, 2):
                                nc.vector.tensor_tensor(
                                    za, za, pt[:, base, :], add)
                                nc.vector.tensor_tensor(
                                    zb, zb, pt[:, base + 1, :], add)
                            # fold the two accumulator chains so the Z
                            # broadcast needs only one PE matmul
                            nc.vector.tensor_tensor(za, za, zb, add)
                            state[(h, c)] = (pt, njb, za)
                        # ---- flush remaining filler ----
                        pull(len(filler))

                # ---------- phase 3: out = yT^T @ WpT ----------
                with (
                    tc.tile_pool(name="wp", bufs=1) as wppool,
                    tc.tile_pool(name="p3ps", bufs=4, space="PSUM") as ps3,
                    tc.tile_pool(name="p3o", bufs=4) as op3,
                ):
                    wp_sb = wppool.tile([128, HG, C], BF16)
                    oq = [nc.sync, nc.scalar]
                    for hh in range(HG):
                        oq[hh % 2].dma_start(
                            out=wp_sb[:, hh, :],
                            in_=wpT[hh * 128:(hh + 1) * 128, :],
                        )
                    for tcb in range(16):
                        pss = [ps3.tile([128, 512], F32, tag="ps3",
                                        name=f"ps3_{cr}")
                               for cr in range(4)]
                        for hh in range(HG):
                            for cr in range(4):
                                nc.tensor.matmul(
                                    pss[cr],
                                    yT_sb[:, hh, tcb * 128:(tcb + 1) * 128],
                                    wp_sb[:, hh, cr * 512:(cr + 1) * 512],
                                    start=(hh == 0),
                                    stop=(hh == HG - 1),
                                )
                        for cr in range(4):
                            ob = op3.tile([128, 512], BF16, tag="ob")
                            if cr % 2 == 0:
                                nc.scalar.activation(ob, pss[cr], Copy)
                            else:
                                nc.vector.tensor_copy(ob, pss[cr])
                            (nc.sync if cr % 2 == 0 else nc.scalar).dma_start(
                                out=out[tcb * 128:(tcb + 1) * 128,
                                        cr * 512:(cr + 1) * 512],
                                in_=ob,
                            )
    nc.compile()
    return nc


def get_nc():
    global _NC_CACHE
    if _NC_CACHE is None:
        _NC_CACHE = _build_program()
    return _NC_CACHE


def prep_core_inputs(inputs):
    """Host-side sharding / layout prep: slice per (b, g), transpose to the
    layouts the device program wants, fold the 1/sqrt(d) softmax scale into
    Wq/bq."""
    f = lambda a: np.asarray(a, dtype=np.float32)
    bf = ml_dtypes.bfloat16
    x = f(inputs["x"])
    am = f(inputs["attn_mask"])
    Wq, bq_ = f(inputs["Wq"]), f(inputs["bq"])
    Wk, bk_ = f(inputs["Wk"]), f(inputs["bk"])
    Wv, bv_ = f(inputs["Wv"]), f(inputs["bv"])
    Wp = f(inputs["Wp"])
    scale = 1.0 / math.sqrt(D)

    # 0/1 staircase in S^T layout: for diagonal block s (0..3) of a 512-wide
    # query chunk, partition p = key offset within the 128-block, column
    # i_local in [0, 512): masked (dead) iff i_local < s*128 + p.
    ii = np.arange(512)[None, :]
    pp = np.arange(128)[:, None]
    cdg01_t = np.stack(
        [np.where(ii < s * 128 + pp, 0.0, 1.0) for s in range(4)], axis=1
    ).astype(bf)  # [128, 4, 512]

    # device DMA layouts: [partition, slice, cc, inner] so every transfer is
    # contiguous per partition
    def to4(wT, ns, ni):
        return np.ascontiguousarray(
            wT.reshape(16, 128, ns, ni).transpose(1, 2, 0, 3)).astype(bf)

    per_g = []
    for g in range(2):
        sl = slice(g * CG, (g + 1) * CG)
        per_g.append(dict(
            wq4=to4(Wq[sl].T * scale, HG, 128),
            wk4=to4(np.ascontiguousarray(Wk[sl].T), HG, 128),
            wv4=to4(np.ascontiguousarray(Wv[sl].T), 4, 256),
            bq=np.ascontiguousarray((bq_[sl] * scale).reshape(HG, 128).T),
            bk=np.ascontiguousarray(bk_[sl].reshape(HG, 128).T),
            bvb=np.ascontiguousarray(
                np.broadcast_to(bv_[sl], (128, CG))
            ).astype(bf),
            wpT=np.ascontiguousarray(Wp[:, sl].T).astype(bf),
        ))

    ones_t = np.ones((128, 128), dtype=bf)

    in_maps = []
    for core in range(N_CORES):
        b, g = core // 2, core % 2
        m = dict(per_g[g])
        m["x4"] = to4(x[b].T, 4, 512)
        m["maskT"] = np.ascontiguousarray(
            am[b, 0, 0, :].reshape(16, 128).T
        )
        m["cdg01"] = cdg01_t
        m["ones128"] = ones_t
        in_maps.append(m)
    return in_maps


def run(inputs, trace=False):
    nc = get_nc()
    in_maps = prep_core_inputs(inputs)
    rr = run_bass_kernel_spmd(nc, in_maps, list(range(N_CORES)), trace=trace)
    bp = np.asarray(inputs["bp"], dtype=np.float32)
    y = np.empty((B, T, C), dtype=np.float32)
    for b in range(B):
        y[b] = (np.asarray(rr.results[2 * b]["out"], dtype=np.float32)
                + np.asarray(rr.results[2 * b + 1]["out"], dtype=np.float32)
                + bp[None, :])
    return y, rr


def kernel(**inputs):
    y, _ = run(inputs)
    return y
